# revision 1
# baseline (speedup 1.0000x reference)
"""TRN2 Bass kernel for FFQLinear: y = x @ ((q - zp) * scale) + bias.

x: [2, 2048, 4096] f32, q: [4096, 4096] int32 (values 0..255),
scale/zero_point: [1] f32, bias: [4096] f32 -> y: [2, 2048, 4096] f32.

Strategy (8 NeuronCores, M split 8 ways, q replicated):
  - Per core: x_shard [512, 4096] f32, q full [4096, 4096] as fp16
    (integers 0..255 are EXACT in fp16), out [512, 4096] f32.
  - The zero-point is handled exactly via a row-sum correction computed
    on the DVE:  y = scale * (x @ q) - (scale*zp) * rowsum(x) + bias
  - Phase 1: transpose the x shard through the PE (fp32 identity matmul)
    into a resident fp16 panel xT [128, 32, 512] (32 KB/partition).
  - Phase 2: stream q in [128, 32, 1024] fp16 pair-panels (64 KB, double
    buffered, 2 KB DMA lines); per panel run 8 PSUM accumulation groups
    STRICTLY SEQUENTIALLY (32 back-to-back matmuls per group - interleaving
    groups across PSUM banks measured 2.8x slower on HW); fused
    scale/zp-rowsum/bias epilogue on DVE.
"""
import numpy as np


def _ensure_paths():
    import sys
    try:
        import concourse  # noqa: F401
        return
    except ImportError:
        pass
    for p in ("/opt/trn_rl_repo", "/root/.axon_site/_ro/trn_rl_repo"):
        if p not in sys.path:
            sys.path.insert(0, p)
    import concourse  # noqa: F401


B, S, DIN, DOUT = 2, 2048, 4096, 4096
N_CORES = 8
M_SH = (B * S) // N_CORES        # 512 rows per core
P = 128
KO = DIN // P                    # 32 k-tiles
MT = M_SH // P                   # 4 m-tiles
NTILE = 512
NPAIR = 1024                     # q panel width (2 n-tiles)
NP = DOUT // NPAIR               # 4 q pair-panels
XCH = 4                          # x load chunks per m-tile
XCW = DIN // XCH                 # 1024 columns per chunk


def _build(scale_f: float, zp_f: float, reps: int = 1, phases=(1, 2)):
    from contextlib import ExitStack
    import concourse.bass as bass
    import concourse.tile as tile
    from concourse import bacc, mybir
    from concourse.masks import make_identity
    from concourse.bass import ts

    f32 = mybir.dt.float32
    f16 = mybir.dt.float16

    nc = bacc.Bacc("TRN2", target_bir_lowering=False, debug=False)

    xs = nc.dram_tensor("xs", [M_SH, DIN], f32, kind="ExternalInput")
    qs = nc.dram_tensor("qs", [DIN, DOUT], f16, kind="ExternalInput")
    biass = nc.dram_tensor("biass", [DOUT], f16, kind="ExternalInput")
    ys = nc.dram_tensor("ys", [M_SH, DOUT], f32, kind="ExternalOutput")

    qs_t = qs.rearrange("(ko p) n -> p ko n", p=P)

    with tile.TileContext(nc) as tc, ExitStack() as ctx:
        const = ctx.enter_context(tc.tile_pool(name="const", bufs=1))
        xt_pool = ctx.enter_context(tc.tile_pool(name="xt_pool", bufs=1))
        q_pool = ctx.enter_context(tc.tile_pool(name="q_pool", bufs=2))
        xs_pool = ctx.enter_context(tc.tile_pool(name="xs_pool", bufs=2))
        y_pool = ctx.enter_context(tc.tile_pool(name="y_pool", bufs=3))
        small = ctx.enter_context(tc.tile_pool(name="small", bufs=2))
        psum = ctx.enter_context(
            tc.tile_pool(name="psum", bufs=8, space="PSUM"))

        ident = const.tile([P, P], f32)
        make_identity(nc, ident)
        bias_sb = const.tile([P, DOUT], f16)
        nc.sync.dma_start(bias_sb[:], biass[:].partition_broadcast(P))

        def body():
            do1, do2 = (1 in phases), (2 in phases)
            # ---- phase 1: transpose x shard into resident fp16 xT panel ----
            # xT[p, ko, m] = x[m, ko*128+p] for this core's m-range
            xT = xt_pool.tile([P, KO, M_SH], f16, tag="xT")
            rs_all = const.tile([P, MT], f32, tag="rs_all")
            if not do1:
                nc.vector.memset(rs_all[:], 0.0)
                nc.vector.memset(xT[:], 0.0)
            for mi in range(MT if do1 else 0):
                rs4 = small.tile([P, XCH], f32, tag="rs4")
                for c in range(XCH):
                    xst = xs_pool.tile([P, XCW], f32, tag="xst")
                    nc.sync.dma_start(xst[:], xs[ts(mi, P), ts(c, XCW)])
                    nc.vector.tensor_reduce(rs4[:, c:c + 1], xst[:],
                                            mybir.AxisListType.X,
                                            mybir.AluOpType.add)
                    for j in range(KO // XCH):
                        ki = (KO // XCH) * c + j
                        tp = psum.tile([P, NTILE], f32, tag="acc")
                        nc.tensor.transpose(tp[:, :P], xst[:, ts(j, P)],
                                            ident[:])
                        if ki % 2 == 0:
                            nc.vector.tensor_copy(out=xT[:, ki, ts(mi, P)],
                                                  in_=tp[:, :P])
                        else:
                            nc.scalar.copy(out=xT[:, ki, ts(mi, P)],
                                           in_=tp[:, :P])
                # rowsum(x m-panel); scaled by -scale*zp below
                nc.vector.tensor_reduce(rs_all[:, mi:mi + 1], rs4[:],
                                        mybir.AxisListType.X,
                                        mybir.AluOpType.add)
            if do1:
                nc.vector.tensor_scalar_mul(rs_all[:], rs_all[:],
                                            -scale_f * zp_f)

            # ---- phase 2: stream q pair-panels, sequential PSUM groups ----
            for np_ in range(NP if do2 else 0):
                qp = q_pool.tile([P, KO, NPAIR], f16, tag="qp")
                nc.sync.dma_start(qp[:], qs_t[:, :, ts(np_, NPAIR)])
                for mi in range(MT):
                    for sub in range(NPAIR // NTILE):
                        acc = psum.tile([P, NTILE], f32, tag="acc",
                                        name=f"acc_{np_}_{mi}_{sub}")
                        for ki in range(KO):
                            nc.tensor.matmul(
                                acc[:], lhsT=xT[:, ki, ts(mi, P)],
                                rhs=qp[:, ki, ts(sub, NTILE)],
                                start=(ki == 0), stop=(ki == KO - 1))
                        ncol = np_ * NPAIR + sub * NTILE
                        y = y_pool.tile([P, NTILE], f32, tag="y")
                        nc.vector.tensor_scalar(y[:], acc[:], scale_f,
                                                rs_all[:, mi:mi + 1],
                                                mybir.AluOpType.mult,
                                                mybir.AluOpType.add)
                        nc.vector.tensor_tensor(
                            y[:], y[:], bias_sb[:, ncol:ncol + NTILE],
                            mybir.AluOpType.add)
                        nc.sync.dma_start(
                            ys[ts(mi, P), ncol:ncol + NTILE], y[:])

        if reps == 1:
            body()
        else:
            with tc.For_i(0, reps, 1):
                body()

    nc.compile()
    return nc


def kernel(x: np.ndarray, q_int_weight: np.ndarray, scale: np.ndarray,
           zero_point: np.ndarray, bias: np.ndarray) -> np.ndarray:
    _ensure_paths()
    from concourse.bass_utils import run_bass_kernel_spmd

    xf = np.ascontiguousarray(x.reshape(B * S, DIN).astype(np.float32))
    scale_f = float(np.asarray(scale).reshape(-1)[0])
    zp_f = float(np.asarray(zero_point).reshape(-1)[0])
    qf = np.ascontiguousarray(q_int_weight.astype(np.float16))  # exact ints
    bf = bias.astype(np.float16)

    nc = _build(scale_f, zp_f)

    in_maps = []
    for c in range(N_CORES):
        in_maps.append({
            "xs": np.ascontiguousarray(xf[c * M_SH:(c + 1) * M_SH]),
            "qs": qf,
            "biass": bf,
        })

    res = run_bass_kernel_spmd(nc, in_maps, core_ids=list(range(N_CORES)))

    y = np.empty((B * S, DOUT), np.float32)
    for c in range(N_CORES):
        y[c * M_SH:(c + 1) * M_SH] = res.results[c]["ys"]
    return y.reshape(B, S, DOUT)



# revision 14
# speedup vs baseline: 183.9192x; 183.9192x over previous
"""TRN2 Bass kernel for FFQLinear: y = x @ ((q - zp) * scale) + bias.

x: [2, 2048, 4096] f32, q: [4096, 4096] int32 (values 0..255),
scale/zero_point: [1] f32, bias: [4096] f32 -> y: [2, 2048, 4096] f32.

Strategy (8 NeuronCores, M split 8 ways, q replicated):
  - Per core: x_shard [512, 4096] f32, q full [4096, 4096] as fp16
    (integers 0..255 are EXACT in fp16), out [512, 4096] f32.
  - The zero-point is handled exactly via a row-sum correction computed
    on the DVE:  y = scale * (x @ q) - (scale*zp) * rowsum(x) + bias
  - x is cast f32->fp16 DURING the load DMA (SWDGE cast, zero engine
    cost), then transposed into per-m-tile panels xT[mi] [128, 32, 128]
    via NORMAL fp16 matmuls against the identity (in_.T @ I ==
    transpose; unlike PE transpose-mode this counts as HAM-warming
    activity, and fp16 streams 1 cycle/row vs 4 for f32). Four 128x128
    transposes land in disjoint 128-col regions of ONE PSUM bank
    (start=True only on the first), evacuated by a single [128,512]
    f32->fp16 DVE copy.
  - Phase 2: stream q in [128, 32, 512] fp16 panels (32 KB, triple
    buffered); per panel run 4 PSUM accumulation groups STRICTLY
    SEQUENTIALLY (32 back-to-back matmuls per group); fused
    scale/zp-rowsum/bias epilogue on DVE.
  - Engine-queue separation so no DMA issue ever head-of-line-blocks
    another stream (critical for cross-iteration pipelining):
      q loads -> sync (SP/HWDGE), x cast-loads -> gpsimd (SWDGE),
      y stores -> scalar (ACT/HWDGE), copies+epilogue -> DVE.
  - Transposes and GEMM use separate 4-bank PSUM pools; all SBUF pools
    are >=2 generations so consecutive iterations overlap.
"""
import numpy as np


def _ensure_paths():
    import sys
    try:
        import concourse  # noqa: F401
        return
    except ImportError:
        pass
    for p in ("/opt/trn_rl_repo", "/root/.axon_site/_ro/trn_rl_repo"):
        if p not in sys.path:
            sys.path.insert(0, p)
    import concourse  # noqa: F401


B, S, DIN, DOUT = 2, 2048, 4096, 4096
N_CORES = 8
M_SH = (B * S) // N_CORES        # 512 rows per core
P = 128
KO = DIN // P                    # 32 k-tiles
MT = M_SH // P                   # 4 m-tiles
NTILE = 512
NP = DOUT // NTILE               # 8 q panels
XCW = 1024                       # x load chunk width
XCH = DIN // XCW                 # 4 x chunks per m-tile
TB = 4                           # transposes batched per PSUM bank


def _build(scale_f: float, zp_f: float, reps: int = 1, phases=(1, 2),
           unroll: int = 1, fences: str = "none"):
    from contextlib import ExitStack
    import concourse.bass as bass
    import concourse.tile as tile
    from concourse import bacc, mybir
    from concourse.masks import make_identity
    from concourse.bass import ts

    f32 = mybir.dt.float32
    f16 = mybir.dt.float16

    nc = bacc.Bacc("TRN2", target_bir_lowering=False, debug=False)

    xs = nc.dram_tensor("xs", [M_SH, DIN], f32, kind="ExternalInput")
    qs = nc.dram_tensor("qs", [DIN, DOUT], f16, kind="ExternalInput")
    biass = nc.dram_tensor("biass", [DOUT], f16, kind="ExternalInput")
    ys = nc.dram_tensor("ys", [M_SH, DOUT], f32, kind="ExternalOutput")

    qs_t = qs.rearrange("(ko p) n -> p ko n", p=P)

    with tile.TileContext(nc) as tc, ExitStack() as ctx:
        const = ctx.enter_context(tc.tile_pool(name="const", bufs=1))
        # per-m-tile xT panels, 2 generations each for cross-iteration overlap
        xt_pools = [
            ctx.enter_context(tc.tile_pool(name=f"xt{mi}", bufs=2))
            for mi in range(MT)
        ]
        rs_pools = [
            ctx.enter_context(tc.tile_pool(name=f"rs{mi}", bufs=2))
            for mi in range(MT)
        ]
        q_pool = ctx.enter_context(tc.tile_pool(name="q_pool", bufs=3))
        xs_pool = ctx.enter_context(tc.tile_pool(name="xs_pool", bufs=4))
        y_pool = ctx.enter_context(tc.tile_pool(name="y_pool", bufs=4))
        small = ctx.enter_context(tc.tile_pool(name="small", bufs=2))
        psum_t = ctx.enter_context(
            tc.tile_pool(name="psum_t", bufs=4, space="PSUM"))
        psum_g = ctx.enter_context(
            tc.tile_pool(name="psum_g", bufs=4, space="PSUM"))

        identf = const.tile([P, P], f32)
        make_identity(nc, identf)
        ident = const.tile([P, P], f16)
        nc.vector.tensor_copy(out=ident[:], in_=identf[:])
        bias_sb = const.tile([P, DOUT], f16)
        nc.sync.dma_start(bias_sb[:], biass[:].partition_broadcast(P))

        def body():
            do1, do2 = (1 in phases), (2 in phases)
            if "xonly" in phases:
                do1 = "xonly"
            # ---- phase 1: transpose x shard into per-m-tile fp16 panels ----
            # xT[mi][p, ko, m] = x[mi*128+m, ko*128+p]
            xts = [xt_pools[mi].tile([P, KO, P], f16, tag=f"xt{mi}",
                                     name=f"xt_{mi}")
                   for mi in range(MT)]
            rss = [rs_pools[mi].tile([P, 1], f32, tag=f"rs{mi}",
                                     name=f"rs_{mi}")
                   for mi in range(MT)]
            if not do1 or do1 == "xonly":
                for mi in range(MT):
                    nc.vector.memset(rss[mi][:], 0.0)
                    nc.vector.memset(xts[mi][:], 0.0)
            for mi in range(MT if do1 else 0):
                rs4 = small.tile([P, XCH], f32, tag="rs4")
                for c in range(XCH):
                    xst = xs_pool.tile([P, XCW], f16, tag="xst")
                    # SWDGE casts f32 -> fp16 inside the DMA engine
                    nc.gpsimd.dma_start(xst[:], xs[ts(mi, P), ts(c, XCW)])
                    nc.vector.tensor_reduce(rs4[:, c:c + 1], xst[:],
                                            mybir.AxisListType.X,
                                            mybir.AluOpType.add)
                    if do1 == "xonly":
                        continue
                    for b in range(XCW // P // TB):
                        tp = psum_t.tile([P, TB * P], f32, tag="tp")
                        for j in range(TB):
                            # normal fp16 matmul vs identity == transpose of
                            # the 128-col slice; start=True only on the first
                            # (clears the bank's has_written bits once, the
                            # rest overwrite their own unwritten region)
                            nc.tensor.matmul(
                                tp[:, ts(j, P)],
                                lhsT=xst[:, ts(b * TB + j, P)],
                                rhs=ident[:],
                                start=(j == 0), stop=(j == TB - 1))
                        kb = (XCW // P) * c + b * TB
                        nc.vector.tensor_copy(
                            out=xts[mi][:, kb:kb + TB, :], in_=tp[:])
                # rowsum(x m-panel), scaled by -scale*zp for the epilogue
                nc.vector.tensor_reduce(rss[mi][:], rs4[:],
                                        mybir.AxisListType.X,
                                        mybir.AluOpType.add)
                nc.vector.tensor_scalar_mul(rss[mi][:], rss[mi][:],
                                            -scale_f * zp_f)

            # Optional scheduler-only fence (no runtime semaphores) to stop
            # the Tile scheduler from splicing phase-1 transpose matmuls
            # into phase-2 PSUM accumulation groups.
            if fences in ("within", "both"):
                tc.no_sync_barrier()

            # ---- phase 2: stream q panels, sequential PSUM groups ----
            for np_ in range(NP if do2 else 0):
                qp = q_pool.tile([P, KO, NTILE], f16, tag="qp")
                nc.sync.dma_start(qp[:], qs_t[:, :, ts(np_, NTILE)])
                for mi in range(MT):
                    acc = psum_g.tile([P, NTILE], f32, tag="acc",
                                      name=f"acc_{np_}_{mi}")
                    for ki in range(KO):
                        nc.tensor.matmul(
                            acc[:], lhsT=xts[mi][:, ki, :],
                            rhs=qp[:, ki, :],
                            start=(ki == 0), stop=(ki == KO - 1))
                    y = y_pool.tile([P, NTILE], f32, tag="y")
                    nc.vector.tensor_scalar(y[:], acc[:], scale_f,
                                            rss[mi][:],
                                            mybir.AluOpType.mult,
                                            mybir.AluOpType.add)
                    nc.vector.tensor_tensor(
                        y[:], y[:], bias_sb[:, ts(np_, NTILE)],
                        mybir.AluOpType.add)
                    nc.scalar.dma_start(
                        ys[ts(mi, P), ts(np_, NTILE)], y[:])

        if reps == 1:
            body()
        else:
            assert reps % unroll == 0
            if reps == unroll:
                for _ in range(reps):
                    body()
                    if fences in ("between", "both"):
                        tc.no_sync_barrier()
            else:
                with tc.For_i(0, reps // unroll, 1):
                    for _ in range(unroll):
                        body()
                        if fences in ("between", "both"):
                            tc.no_sync_barrier()

    nc.compile()
    return nc


def kernel(x: np.ndarray, q_int_weight: np.ndarray, scale: np.ndarray,
           zero_point: np.ndarray, bias: np.ndarray) -> np.ndarray:
    _ensure_paths()
    from concourse.bass_utils import run_bass_kernel_spmd

    xf = np.ascontiguousarray(x.reshape(B * S, DIN).astype(np.float32))
    scale_f = float(np.asarray(scale).reshape(-1)[0])
    zp_f = float(np.asarray(zero_point).reshape(-1)[0])
    qf = np.ascontiguousarray(q_int_weight.astype(np.float16))  # exact ints
    bf = bias.astype(np.float16)

    nc = _build(scale_f, zp_f)

    in_maps = []
    for c in range(N_CORES):
        in_maps.append({
            "xs": np.ascontiguousarray(xf[c * M_SH:(c + 1) * M_SH]),
            "qs": qf,
            "biass": bf,
        })

    res = run_bass_kernel_spmd(nc, in_maps, core_ids=list(range(N_CORES)))

    y = np.empty((B * S, DOUT), np.float32)
    for c in range(N_CORES):
        y[c * M_SH:(c + 1) * M_SH] = res.results[c]["ys"]
    return y.reshape(B, S, DOUT)


# revision 16
# speedup vs baseline: 216.2570x; 1.1758x over previous
"""TRN2 Bass kernel for FFQLinear: y = x @ ((q - zp) * scale) + bias.

x: [2, 2048, 4096] f32, q: [4096, 4096] int32 (values 0..255),
scale/zero_point: [1] f32, bias: [4096] f32 -> y: [2, 2048, 4096] f32.

Strategy (8 NeuronCores, M split 8 ways, q replicated):
  - Per core: x_shard [512, 4096] f32, q full [4096, 4096] as fp16
    (integers 0..255 are EXACT in fp16), out [512, 4096] f32.
  - The zero-point is handled exactly via a row-sum correction computed
    on the DVE:  y = scale * (x @ q) - (scale*zp) * rowsum(x) + bias
  - x is cast f32->fp16 DURING the load DMA (SWDGE cast, zero engine
    cost), then transposed into per-m-tile panels xT[mi] [128, 32, 128]
    via NORMAL fp16 matmuls against the identity (in_.T @ I ==
    transpose; unlike PE transpose-mode this counts as HAM-warming
    activity, and fp16 streams 1 cycle/row vs 4 for f32). Four 128x128
    transposes land in disjoint 128-col regions of ONE PSUM bank
    (start=True only on the first), evacuated by a single [128,512]
    f32->fp16 DVE copy.
  - Phase 2: stream q in [128, 32, 512] fp16 panels (32 KB, triple
    buffered); per panel run 4 PSUM accumulation groups STRICTLY
    SEQUENTIALLY (32 back-to-back matmuls per group); fused
    scale/zp-rowsum/bias epilogue on DVE.
  - Engine-queue separation so no DMA issue ever head-of-line-blocks
    another stream (critical for cross-iteration pipelining):
      q loads -> sync (SP/HWDGE), x cast-loads -> gpsimd (SWDGE),
      y stores -> scalar (ACT/HWDGE), copies+epilogue -> DVE.
  - Transposes and GEMM use separate 4-bank PSUM pools; all SBUF pools
    are >=2 generations so consecutive iterations overlap.
"""
import numpy as np


def _ensure_paths():
    import sys
    try:
        import concourse  # noqa: F401
        return
    except ImportError:
        pass
    for p in ("/opt/trn_rl_repo", "/root/.axon_site/_ro/trn_rl_repo"):
        if p not in sys.path:
            sys.path.insert(0, p)
    import concourse  # noqa: F401


B, S, DIN, DOUT = 2, 2048, 4096, 4096
N_CORES = 8
M_SH = (B * S) // N_CORES        # 512 rows per core
P = 128
KO = DIN // P                    # 32 k-tiles
MT = M_SH // P                   # 4 m-tiles
NTILE = 512
NP = DOUT // NTILE               # 8 q panels
XCW = 2048                       # x load chunk width
XCH = DIN // XCW                 # 2 x chunks per m-tile
TB = 4                           # transposes batched per PSUM bank


def _build(scale_f: float, zp_f: float, reps: int = 1, phases=(1, 2),
           unroll: int = 1, fences: str = "none"):
    from contextlib import ExitStack
    import concourse.bass as bass
    import concourse.tile as tile
    from concourse import bacc, mybir
    from concourse.masks import make_identity
    from concourse.bass import ts

    f32 = mybir.dt.float32
    f16 = mybir.dt.float16

    nc = bacc.Bacc("TRN2", target_bir_lowering=False, debug=False)

    xs = nc.dram_tensor("xs", [M_SH, DIN], f32, kind="ExternalInput")
    qs = nc.dram_tensor("qs", [DIN, DOUT], f16, kind="ExternalInput")
    biass = nc.dram_tensor("biass", [DOUT], f16, kind="ExternalInput")
    ys = nc.dram_tensor("ys", [M_SH, DOUT], f32, kind="ExternalOutput")

    qs_t = qs.rearrange("(ko p) n -> p ko n", p=P)

    with tile.TileContext(nc) as tc, ExitStack() as ctx:
        const = ctx.enter_context(tc.tile_pool(name="const", bufs=1))
        # per-m-tile xT panels, 2 generations each for cross-iteration overlap
        xt_pools = [
            ctx.enter_context(tc.tile_pool(name=f"xt{mi}", bufs=2))
            for mi in range(MT)
        ]
        rs_pools = [
            ctx.enter_context(tc.tile_pool(name=f"rs{mi}", bufs=2))
            for mi in range(MT)
        ]
        q_pool = ctx.enter_context(tc.tile_pool(name="q_pool", bufs=3))
        xs_pool = ctx.enter_context(tc.tile_pool(name="xs_pool", bufs=3))
        y_pool = ctx.enter_context(tc.tile_pool(name="y_pool", bufs=4))
        small = ctx.enter_context(tc.tile_pool(name="small", bufs=2))
        psum_t = ctx.enter_context(
            tc.tile_pool(name="psum_t", bufs=4, space="PSUM"))
        psum_g = ctx.enter_context(
            tc.tile_pool(name="psum_g", bufs=4, space="PSUM"))

        identf = const.tile([P, P], f32)
        make_identity(nc, identf)
        ident = const.tile([P, P], f16)
        nc.vector.tensor_copy(out=ident[:], in_=identf[:])
        bias_sb = const.tile([P, DOUT], f16)
        nc.sync.dma_start(bias_sb[:], biass[:].partition_broadcast(P))

        def body():
            do1, do2 = (1 in phases), (2 in phases)
            if "xonly" in phases:
                do1 = "xonly"
            # ---- phase 1: transpose x shard into per-m-tile fp16 panels ----
            # xT[mi][p, ko, m] = x[mi*128+m, ko*128+p]
            xts = [xt_pools[mi].tile([P, KO, P], f16, tag=f"xt{mi}",
                                     name=f"xt_{mi}")
                   for mi in range(MT)]
            rss = [rs_pools[mi].tile([P, 1], f32, tag=f"rs{mi}",
                                     name=f"rs_{mi}")
                   for mi in range(MT)]
            if not do1 or do1 == "xonly":
                for mi in range(MT):
                    nc.vector.memset(rss[mi][:], 0.0)
                    nc.vector.memset(xts[mi][:], 0.0)
            for mi in range(MT if do1 else 0):
                rs4 = small.tile([P, XCH], f32, tag="rs4")
                for c in range(XCH):
                    xst = xs_pool.tile([P, XCW], f16, tag="xst")
                    # SWDGE casts f32 -> fp16 inside the DMA engine
                    nc.gpsimd.dma_start(xst[:], xs[ts(mi, P), ts(c, XCW)])
                    nc.vector.tensor_reduce(rs4[:, c:c + 1], xst[:],
                                            mybir.AxisListType.X,
                                            mybir.AluOpType.add)
                    if do1 == "xonly":
                        continue
                    for b in range(XCW // P // TB):
                        tp = psum_t.tile([P, TB * P], f32, tag="tp")
                        for j in range(TB):
                            # normal fp16 matmul vs identity == transpose of
                            # the 128-col slice; start=True only on the first
                            # (clears the bank's has_written bits once, the
                            # rest overwrite their own unwritten region)
                            nc.tensor.matmul(
                                tp[:, ts(j, P)],
                                lhsT=xst[:, ts(b * TB + j, P)],
                                rhs=ident[:],
                                start=(j == 0), stop=(j == TB - 1))
                        kb = (XCW // P) * c + b * TB
                        nc.vector.tensor_copy(
                            out=xts[mi][:, kb:kb + TB, :], in_=tp[:])
                # rowsum(x m-panel), scaled by -scale*zp for the epilogue
                nc.vector.tensor_reduce(rss[mi][:], rs4[:],
                                        mybir.AxisListType.X,
                                        mybir.AluOpType.add)
                nc.vector.tensor_scalar_mul(rss[mi][:], rss[mi][:],
                                            -scale_f * zp_f)

            # Optional scheduler-only fence (no runtime semaphores) to stop
            # the Tile scheduler from splicing phase-1 transpose matmuls
            # into phase-2 PSUM accumulation groups.
            if fences in ("within", "both"):
                tc.no_sync_barrier()

            # ---- phase 2: stream q panels, sequential PSUM groups ----
            for np_ in range(NP if do2 else 0):
                qp = q_pool.tile([P, KO, NTILE], f16, tag="qp")
                nc.sync.dma_start(qp[:], qs_t[:, :, ts(np_, NTILE)])
                for mi in range(MT):
                    acc = psum_g.tile([P, NTILE], f32, tag="acc",
                                      name=f"acc_{np_}_{mi}")
                    for ki in range(KO):
                        nc.tensor.matmul(
                            acc[:], lhsT=xts[mi][:, ki, :],
                            rhs=qp[:, ki, :],
                            start=(ki == 0), stop=(ki == KO - 1))
                    y = y_pool.tile([P, NTILE], f32, tag="y")
                    nc.vector.tensor_scalar(y[:], acc[:], scale_f,
                                            rss[mi][:],
                                            mybir.AluOpType.mult,
                                            mybir.AluOpType.add)
                    nc.vector.tensor_tensor(
                        y[:], y[:], bias_sb[:, ts(np_, NTILE)],
                        mybir.AluOpType.add)
                    nc.scalar.dma_start(
                        ys[ts(mi, P), ts(np_, NTILE)], y[:])

        if reps == 1:
            body()
        else:
            assert reps % unroll == 0
            if reps == unroll:
                for _ in range(reps):
                    body()
                    if fences in ("between", "both"):
                        tc.no_sync_barrier()
            else:
                with tc.For_i(0, reps // unroll, 1):
                    for _ in range(unroll):
                        body()
                        if fences in ("between", "both"):
                            tc.no_sync_barrier()

    nc.compile()
    return nc


def kernel(x: np.ndarray, q_int_weight: np.ndarray, scale: np.ndarray,
           zero_point: np.ndarray, bias: np.ndarray) -> np.ndarray:
    _ensure_paths()
    from concourse.bass_utils import run_bass_kernel_spmd

    xf = np.ascontiguousarray(x.reshape(B * S, DIN).astype(np.float32))
    scale_f = float(np.asarray(scale).reshape(-1)[0])
    zp_f = float(np.asarray(zero_point).reshape(-1)[0])
    qf = np.ascontiguousarray(q_int_weight.astype(np.float16))  # exact ints
    bf = bias.astype(np.float16)

    nc = _build(scale_f, zp_f)

    in_maps = []
    for c in range(N_CORES):
        in_maps.append({
            "xs": np.ascontiguousarray(xf[c * M_SH:(c + 1) * M_SH]),
            "qs": qf,
            "biass": bf,
        })

    res = run_bass_kernel_spmd(nc, in_maps, core_ids=list(range(N_CORES)))

    y = np.empty((B * S, DOUT), np.float32)
    for c in range(N_CORES):
        y[c * M_SH:(c + 1) * M_SH] = res.results[c]["ys"]
    return y.reshape(B, S, DOUT)


# revision 19
# speedup vs baseline: 223.8589x; 1.0352x over previous
"""TRN2 Bass kernel for FFQLinear: y = x @ ((q - zp) * scale) + bias.

x: [2, 2048, 4096] f32, q: [4096, 4096] int32 (values 0..255),
scale/zero_point: [1] f32, bias: [4096] f32 -> y: [2, 2048, 4096] f32.

Strategy (8 NeuronCores, M split 8 ways, q replicated):
  - Per core: x_shard [512, 4096] f32, q full [4096, 4096] as fp16
    (integers 0..255 are EXACT in fp16), out [512, 4096] f32.
  - The zero-point is handled exactly via a row-sum correction computed
    on the DVE:  y = scale * (x @ q) - (scale*zp) * rowsum(x) + bias
  - x is cast f32->fp16 DURING the load DMA (SWDGE cast, zero engine
    cost), then transposed into per-m-tile panels xT[mi] [128, 32, 128]
    via NORMAL fp16 matmuls against the identity (in_.T @ I ==
    transpose; unlike PE transpose-mode this counts as HAM-warming
    activity, and fp16 streams 1 cycle/row vs 4 for f32). Four 128x128
    transposes land in disjoint 128-col regions of ONE PSUM bank
    (start=True only on the first), evacuated by a single [128,512]
    f32->fp16 DVE copy.
  - Phase 2: stream q in [128, 32, 512] fp16 panels (32 KB, triple
    buffered); per panel run 4 PSUM accumulation groups STRICTLY
    SEQUENTIALLY (32 back-to-back matmuls per group); fused
    scale/zp-rowsum/bias epilogue on DVE.
  - Engine-queue separation so no DMA issue ever head-of-line-blocks
    another stream (critical for cross-iteration pipelining):
      q loads -> sync (SP/HWDGE), x cast-loads -> gpsimd (SWDGE),
      y stores -> scalar (ACT/HWDGE), copies+epilogue -> DVE.
  - Transposes and GEMM use separate 4-bank PSUM pools; all SBUF pools
    are >=2 generations so consecutive iterations overlap.
"""
import numpy as np


def _ensure_paths():
    import sys
    try:
        import concourse  # noqa: F401
        return
    except ImportError:
        pass
    for p in ("/opt/trn_rl_repo", "/root/.axon_site/_ro/trn_rl_repo"):
        if p not in sys.path:
            sys.path.insert(0, p)
    import concourse  # noqa: F401


B, S, DIN, DOUT = 2, 2048, 4096, 4096
N_CORES = 8
M_SH = (B * S) // N_CORES        # 512 rows per core
P = 128
KO = DIN // P                    # 32 k-tiles
MT = M_SH // P                   # 4 m-tiles
NTILE = 512
NP = DOUT // NTILE               # 8 q panels
XCW = 2048                       # x load chunk width
XCH = DIN // XCW                 # 2 x chunks per m-tile
TB = 8                           # transposes batched per PSUM tile (2 banks)


def _build(scale_f: float, zp_f: float, reps: int = 1, phases=(1, 2),
           unroll: int = 1, fences: str = "none"):
    from contextlib import ExitStack
    import concourse.bass as bass
    import concourse.tile as tile
    from concourse import bacc, mybir
    from concourse.masks import make_identity
    from concourse.bass import ts

    f32 = mybir.dt.float32
    f16 = mybir.dt.float16

    nc = bacc.Bacc("TRN2", target_bir_lowering=False, debug=False)

    xs = nc.dram_tensor("xs", [M_SH, DIN], f32, kind="ExternalInput")
    qs = nc.dram_tensor("qs", [DIN, DOUT], f16, kind="ExternalInput")
    biass = nc.dram_tensor("biass", [DOUT], f16, kind="ExternalInput")
    ys = nc.dram_tensor("ys", [M_SH, DOUT], f32, kind="ExternalOutput")

    qs_t = qs.rearrange("(ko p) n -> p ko n", p=P)

    with tile.TileContext(nc) as tc, ExitStack() as ctx:
        const = ctx.enter_context(tc.tile_pool(name="const", bufs=1))
        # per-m-tile xT panels, 2 generations each for cross-iteration overlap
        xt_pools = [
            ctx.enter_context(tc.tile_pool(name=f"xt{mi}", bufs=2))
            for mi in range(MT)
        ]
        rs_pools = [
            ctx.enter_context(tc.tile_pool(name=f"rs{mi}", bufs=2))
            for mi in range(MT)
        ]
        q_pool = ctx.enter_context(tc.tile_pool(name="q_pool", bufs=3))
        xs_pool = ctx.enter_context(tc.tile_pool(name="xs_pool", bufs=3))
        y_pool = ctx.enter_context(tc.tile_pool(name="y_pool", bufs=4))
        small = ctx.enter_context(tc.tile_pool(name="small", bufs=2))
        psum_t = ctx.enter_context(
            tc.tile_pool(name="psum_t", bufs=2, space="PSUM"))
        psum_g = ctx.enter_context(
            tc.tile_pool(name="psum_g", bufs=4, space="PSUM"))

        identf = const.tile([P, P], f32)
        make_identity(nc, identf)
        ident = const.tile([P, P], f16)
        nc.vector.tensor_copy(out=ident[:], in_=identf[:])
        bias_sb = const.tile([P, DOUT], f16)
        nc.sync.dma_start(bias_sb[:], biass[:].partition_broadcast(P))

        def body():
            do1, do2 = (1 in phases), (2 in phases)
            if "xonly" in phases:
                do1 = "xonly"
            # ---- phase 1: transpose x shard into per-m-tile fp16 panels ----
            # xT[mi][p, ko, m] = x[mi*128+m, ko*128+p]
            xts = [xt_pools[mi].tile([P, KO, P], f16, tag=f"xt{mi}",
                                     name=f"xt_{mi}")
                   for mi in range(MT)]
            rss = [rs_pools[mi].tile([P, 1], f32, tag=f"rs{mi}",
                                     name=f"rs_{mi}")
                   for mi in range(MT)]
            if not do1 or do1 == "xonly":
                for mi in range(MT):
                    nc.vector.memset(rss[mi][:], 0.0)
                    nc.vector.memset(xts[mi][:], 0.0)
            for mi in range(MT if do1 else 0):
                rs4 = small.tile([P, XCH], f32, tag="rs4")
                for c in range(XCH):
                    xst = xs_pool.tile([P, XCW], f16, tag="xst")
                    # SWDGE casts f32 -> fp16 inside the DMA engine
                    nc.gpsimd.dma_start(xst[:], xs[ts(mi, P), ts(c, XCW)])
                    nc.vector.tensor_reduce(rs4[:, c:c + 1], xst[:],
                                            mybir.AxisListType.X,
                                            mybir.AluOpType.add)
                    if do1 == "xonly":
                        continue
                    for b in range(XCW // P // TB):
                        tp = psum_t.tile([P, TB * P], f32, tag="tp")
                        for j in range(TB):
                            # normal fp16 matmul vs identity == transpose of
                            # the 128-col slice; start=True once per 2KB PSUM
                            # bank (clears that bank's has_written bits; the
                            # rest overwrite their own unwritten region)
                            nc.tensor.matmul(
                                tp[:, ts(j, P)],
                                lhsT=xst[:, ts(b * TB + j, P)],
                                rhs=ident[:],
                                start=(j % 4 == 0), stop=(j % 4 == 3))
                        kb = (XCW // P) * c + b * TB
                        nc.vector.tensor_copy(
                            out=xts[mi][:, kb:kb + TB, :], in_=tp[:])
                # rowsum(x m-panel), scaled by -scale*zp for the epilogue
                nc.vector.tensor_reduce(rss[mi][:], rs4[:],
                                        mybir.AxisListType.X,
                                        mybir.AluOpType.add)
                nc.vector.tensor_scalar_mul(rss[mi][:], rss[mi][:],
                                            -scale_f * zp_f)

            # Optional scheduler-only fence (no runtime semaphores) to stop
            # the Tile scheduler from splicing phase-1 transpose matmuls
            # into phase-2 PSUM accumulation groups.
            if fences in ("within", "both"):
                tc.no_sync_barrier()

            # ---- phase 2: stream q panels, sequential PSUM groups ----
            for np_ in range(NP if do2 else 0):
                qp = q_pool.tile([P, KO, NTILE], f16, tag="qp")
                nc.sync.dma_start(qp[:], qs_t[:, :, ts(np_, NTILE)])
                for mi in range(MT):
                    acc = psum_g.tile([P, NTILE], f32, tag="acc",
                                      name=f"acc_{np_}_{mi}")
                    for ki in range(KO):
                        nc.tensor.matmul(
                            acc[:], lhsT=xts[mi][:, ki, :],
                            rhs=qp[:, ki, :],
                            start=(ki == 0), stop=(ki == KO - 1))
                    y = y_pool.tile([P, NTILE], f32, tag="y")
                    nc.vector.tensor_scalar(y[:], acc[:], scale_f,
                                            rss[mi][:],
                                            mybir.AluOpType.mult,
                                            mybir.AluOpType.add)
                    nc.vector.tensor_tensor(
                        y[:], y[:], bias_sb[:, ts(np_, NTILE)],
                        mybir.AluOpType.add)
                    nc.scalar.dma_start(
                        ys[ts(mi, P), ts(np_, NTILE)], y[:])

        if reps == 1:
            body()
        else:
            assert reps % unroll == 0
            if reps == unroll:
                for _ in range(reps):
                    body()
                    if fences in ("between", "both"):
                        tc.no_sync_barrier()
            else:
                with tc.For_i(0, reps // unroll, 1):
                    for _ in range(unroll):
                        body()
                        if fences in ("between", "both"):
                            tc.no_sync_barrier()

    nc.compile()
    return nc


def kernel(x: np.ndarray, q_int_weight: np.ndarray, scale: np.ndarray,
           zero_point: np.ndarray, bias: np.ndarray) -> np.ndarray:
    _ensure_paths()
    from concourse.bass_utils import run_bass_kernel_spmd

    xf = np.ascontiguousarray(x.reshape(B * S, DIN).astype(np.float32))
    scale_f = float(np.asarray(scale).reshape(-1)[0])
    zp_f = float(np.asarray(zero_point).reshape(-1)[0])
    qf = np.ascontiguousarray(q_int_weight.astype(np.float16))  # exact ints
    bf = bias.astype(np.float16)

    nc = _build(scale_f, zp_f)

    in_maps = []
    for c in range(N_CORES):
        in_maps.append({
            "xs": np.ascontiguousarray(xf[c * M_SH:(c + 1) * M_SH]),
            "qs": qf,
            "biass": bf,
        })

    res = run_bass_kernel_spmd(nc, in_maps, core_ids=list(range(N_CORES)))

    y = np.empty((B * S, DOUT), np.float32)
    for c in range(N_CORES):
        y[c * M_SH:(c + 1) * M_SH] = res.results[c]["ys"]
    return y.reshape(B, S, DOUT)
